# revision 21
# baseline (speedup 1.0000x reference)
"""Trainium2 Bass kernel for nn_Attention_65455301591248.

Multi-head attention: B=32, C=768, H=12 heads, S=512, D=64.
  q/k/v = W{q,k,v} @ x + b   (1x1 conv == channel GEMM), head-minor channel
  scores[k,h,q] = (q.k)/sqrt(D) + mask[k,q];  softmax over k
  attn = w @ v; concat head-major; out = Wo @ attn + bo

Sharding: pure data parallel over batch - 4 batches per core x 8 cores,
no collectives.

Per-core kernel strategy (v2):
  - Host pre-transposes weights (lhsT layout) and permutes Q/K/V output
    channels head-major (c' = h*64+d).  Wq/bq pre-scaled by 1/8.
    bv folded into bo on host (softmax weights sum to 1, so +bv passes
    through attention unchanged): bo' = bo + Wo @ bv_hm.  Exact.
  - Scores for a head PAIR go into one [128,1024] PSUM tile (2 banks):
    even head -> cols 0:512 / rows 0:63 of the PE array, odd head ->
    cols 512:1024 / rows 64:127, explicit tile_position so the two
    K=64 matmuls run CONCURRENTLY (row tiling).
  - One exp per (pair,kc) over [128,1024] on ACT; one DVE multiply by
    em2 (mask exp, column-duplicated so the operand is contiguous).
    em is scaled by 2^-6 so fp16 reciprocals / weights stay in normal
    range; the scale cancels between numerator and denominator.
  - attn = V^T-slab @ w with a ones column per head: PSUM row 64
    accumulates the softmax denominator for free.  Eviction writes all
    heads of a half-batch into one [65,3072] tile, so ONE gather DMA
    collects the 6 denominator rows; reciprocal via
    reciprocal_approx_fast (fp32); one scatter DMA -> partition-0
    staging; gpsimd partition_broadcast + fp16 2x-mode DVE multiplies
    normalize into the head-major concat buffer.
  - Single large DMAs for weights / x / mask; prologue orders
    wq,wk,x,mask before wv,wo so score matmuls start early and the PE
    clock (HAM) stays warm.
"""

import numpy as np

try:
    import concourse.bass as bass  # noqa: F401
except ImportError:  # pragma: no cover
    import sys

    sys.path.insert(0, "/opt/trn_rl_repo")

import concourse.bass as bass
import concourse.tile as tile
from concourse import bacc, mybir
from concourse.bass_utils import run_bass_kernel_spmd

B, C, H, S, D = 32, 768, 12, 512, 64
NCORES = 8
NB = B // NCORES  # batches per core
F16 = mybir.dt.float16
F32 = mybir.dt.float32
NC_CHUNKS = C // 128  # 6
NK_CHUNKS = S // 128  # 4
VROW = H * (D + 1)  # 780: per-head 64 v columns + 1 ones column
EM_BIAS = -6.0 * float(np.log(2.0))  # exp(mask)*2^-6

_COMPILED = None


def _build():
    """Build + compile the per-core Bass program (runs on each of 8 cores)."""
    nc = bacc.Bacc("TRN2", target_bir_lowering=False, debug=False)

    x_d = nc.dram_tensor("x", [NB, C, S], F16, kind="ExternalInput")
    m_d = nc.dram_tensor("mask", [NB, S, S], F16, kind="ExternalInput")
    wq_d = nc.dram_tensor("wqt", [C, C], F16, kind="ExternalInput")
    wk_d = nc.dram_tensor("wkt", [C, C], F16, kind="ExternalInput")
    wv_d = nc.dram_tensor("wvt", [C, C], F16, kind="ExternalInput")
    wo_d = nc.dram_tensor("wot", [C, C], F16, kind="ExternalInput")
    # packed per-partition biases: cols 0-5 bq/8, 6-11 bk, 12-17 bo',
    # col 18 = EM_BIAS constant
    bcol_d = nc.dram_tensor("bcols", [128, 19], F32, kind="ExternalInput")
    y_d = nc.dram_tensor("y", [NB, C, S], F16, kind="ExternalOutput")

    with tile.TileContext(nc) as tc:
        with (
            tc.tile_pool(name="wpool", bufs=1) as wpool,
            tc.tile_pool(name="const", bufs=1) as const,
            tc.tile_pool(name="xp", bufs=2) as xp,
            tc.tile_pool(name="qk", bufs=2) as qk,
            tc.tile_pool(name="vp", bufs=2) as vp,
            tc.tile_pool(name="mp", bufs=2) as mp,
            tc.tile_pool(name="wexp", bufs=2) as wexp,
            tc.tile_pool(name="stgp", bufs=2) as stgp,
            tc.tile_pool(name="cat", bufs=2) as cat,
            tc.tile_pool(name="op", bufs=2) as op,
            tc.tile_pool(name="rp", bufs=2) as rp,
            tc.tile_pool(name="ps_proj", bufs=2, space="PSUM") as ps_proj,
            tc.tile_pool(name="ps_s", bufs=2, space="PSUM") as ps_s,
            tc.tile_pool(name="ps_a", bufs=2, space="PSUM") as ps_a,
        ):
            # ---- persistent weights / constants -------------------------
            def load_w(w_d, name):
                t = wpool.tile([128, NC_CHUNKS * C], F16, tag=name, name=name)
                nc.sync.dma_start(
                    out=t.rearrange("p (j c) -> p j c", c=C),
                    in_=w_d.ap().rearrange("(j p) c -> p j c", p=128),
                )
                return t

            bcol = const.tile([128, 19], F32, tag="bcol")
            nc.sync.dma_start(out=bcol[:], in_=bcol_d.ap()[:, :])

            def wview(t, ki):
                return t[:, ki * C : (ki + 1) * C]

            # ---- per-batch input loads ---------------------------------
            def load_x(b):
                t = xp.tile([128, NC_CHUNKS * S], F16, tag="x", name=f"x{b}")
                nc.sync.dma_start(
                    out=t.rearrange("p (j s) -> p j s", s=S),
                    in_=x_d.ap()[b].rearrange("(j p) s -> p j s", p=128),
                )
                return t

            def load_mask(b):
                t = mp.tile([128, NK_CHUNKS * S], F16, tag="mraw", name=f"m{b}")
                nc.sync.dma_start(
                    out=t.rearrange("p (kc q) -> p kc q", q=S),
                    in_=m_d.ap()[b].rearrange("(kc p) q -> p kc q", p=128),
                )
                return t

            def em_thunk(mt, em_out):
                def one():
                    # in-place exp: em = exp(mask + EM_BIAS) = exp(mask)*2^-6
                    nc.scalar.activation(
                        out=mt[:],
                        in_=mt[:],
                        func=mybir.ActivationFunctionType.Exp,
                        bias=bcol[:, 18:19],
                    )
                    em_out[0] = mt

                return one

            # ---- projection groups -------------------------------------
            def qk_group(w_t, xt, bias_col, name, co, outs):
                ps = ps_proj.tile([128, S], F32, tag="proj", name="ps_p")
                for ki in range(NC_CHUNKS):
                    nc.tensor.matmul(
                        ps[:],
                        wview(w_t, ki)[:, co * 128 : (co + 1) * 128],
                        xt[:, ki * S : (ki + 1) * S],
                        start=(ki == 0),
                        stop=(ki == NC_CHUNKS - 1),
                    )
                dt = qk.tile([128, S], F16, tag=f"{name}{co}", name=f"{name}{co}")
                if co % 2 == 0:
                    nc.vector.tensor_scalar_add(
                        dt[:], ps[:], bcol[:, bias_col + co : bias_col + co + 1]
                    )
                else:
                    nc.scalar.activation(
                        out=dt[:],
                        in_=ps[:],
                        func=mybir.ActivationFunctionType.Identity,
                        bias=bcol[:, bias_col + co : bias_col + co + 1],
                    )
                outs[co] = dt

            def v_group(wv_t, xt, sc, half, v_out):
                # v^T projection chunk: out [s, c'] with per-head ones col
                if half == 0:
                    vt = vp.tile([128, VROW], F16, tag=f"v{sc}", name=f"v{sc}")
                    vv = vt.rearrange("p (h w) -> p h w", w=D + 1)
                    nc.vector.memset(vv[:, :, D : D + 1], 1.0)
                    v_out[sc] = vt
                else:
                    vt = v_out[sc]
                    vv = vt.rearrange("p (h w) -> p h w", w=D + 1)
                v_out[(sc, half)] = True
                hw = C // 2  # 384 = 6 heads
                ps = ps_proj.tile([128, hw], F32, tag="proj", name="ps_v")
                for ki in range(NC_CHUNKS):
                    nc.tensor.matmul(
                        ps[:],
                        xt[:, ki * S + sc * 128 : ki * S + (sc + 1) * 128],
                        wview(wv_t, ki)[:, half * hw : (half + 1) * hw],
                        start=(ki == 0),
                        stop=(ki == NC_CHUNKS - 1),
                    )
                if half == 0:
                    nc.scalar.activation(
                        out=vv[:, 0:6, 0:D],
                        in_=ps.rearrange("p (h w) -> p h w", w=D),
                        func=mybir.ActivationFunctionType.Copy,
                    )
                else:
                    nc.vector.tensor_copy(
                        vv[:, 6:12, 0:D],
                        ps.rearrange("p (h w) -> p h w", w=D),
                    )

            def qkv_thunks(wv_load, xt, q_out, k_out, v_out):
                th = []
                for co in range(NC_CHUNKS):
                    th.append(lambda co=co: qk_group(wq_t, xt, 0, "q", co, q_out))
                    th.append(lambda co=co: qk_group(wk_t, xt, 6, "k", co, k_out))
                for sc in range(NK_CHUNKS):
                    for half in range(2):
                        th.append(
                            lambda sc=sc, half=half: v_group(
                                wv_load[0], xt, sc, half, v_out
                            )
                        )
                return th

            def oproj_thunks(wo_load, b, cat_sb):
                def one(co):
                    ps = ps_proj.tile([128, S], F32, tag="proj", name="ps_o")
                    for ki in range(NC_CHUNKS):
                        nc.tensor.matmul(
                            ps[:],
                            wview(wo_load[0], ki)[:, co * 128 : (co + 1) * 128],
                            cat_sb[ki][:],
                            start=(ki == 0),
                            stop=(ki == NC_CHUNKS - 1),
                        )
                    ot = op.tile([128, S], F16, tag=f"o{co}", name="ot")
                    if co % 2 == 0:
                        nc.vector.tensor_scalar_add(
                            ot[:], ps[:], bcol[:, 12 + co : 13 + co]
                        )
                    else:
                        nc.scalar.activation(
                            out=ot[:],
                            in_=ps[:],
                            func=mybir.ActivationFunctionType.Identity,
                            bias=bcol[:, 12 + co : 13 + co],
                        )
                    nc.sync.dma_start(
                        out=y_d.ap()[b, co * 128 : (co + 1) * 128, :], in_=ot[:]
                    )

                return [lambda co=co: one(co) for co in range(NC_CHUNKS)]

            # ---- attention for one batch -------------------------------
            def attention(b, q_sb, k_sb, v_sb, em_t, work):
                # q_sb/k_sb/v_sb are dicts filled lazily by work thunks;
                # drain_until pulls work forward when an operand tile has
                # not been emitted yet (only matters for batch 0).
                cat_sb = []
                for j in range(NC_CHUNKS):
                    ct = cat.tile([128, S], F16, tag=f"c{j}", name=f"cat{j}")
                    cat_sb.append(ct)
                stg = [
                    stgp.tile([D + 1, 6 * S], F16, tag="stg0", name="stg0"),
                    stgp.tile([D + 1, 6 * S], F16, tag="stg1", name="stg1"),
                ]

                nwork = len(work)
                wi = 0

                def drain_until(cond):
                    nonlocal wi
                    while not cond():
                        assert wi < nwork, "work list exhausted before operand ready"
                        work[wi]()
                        wi += 1

                def emit_scores_pair(hp):
                    em = em_t[0]
                    es_tiles = []
                    for kc in range(NK_CHUNKS):
                        ps = ps_s.tile([128, 2 * S], F32, tag="spair", name="ps_sc")
                        for j in range(2):
                            po = j * D
                            nc.tensor.matmul(
                                ps[:, j * S : (j + 1) * S],
                                k_sb[hp][po : po + D, kc * 128 : (kc + 1) * 128],
                                q_sb[hp][po : po + D, :],
                                start=True,
                                stop=True,
                                tile_position=(po, 0),
                            )
                        es = wexp.tile([128, 2 * S], F16, tag=f"es{kc}", name="es")
                        nc.scalar.activation(
                            out=es[:],
                            in_=ps[:],
                            func=mybir.ActivationFunctionType.Exp,
                        )
                        # multiply both head-halves by em[kc] via 0-stride
                        # broadcast of the mask-exp slice
                        nc.vector.tensor_mul(
                            es.rearrange("p (r q) -> p r q", r=2),
                            es.rearrange("p (r q) -> p r q", r=2),
                            em[:, kc * S : (kc + 1) * S]
                            .unsqueeze(1)
                            .broadcast_to([128, 2, S]),
                        )
                        es_tiles.append(es)
                    return es_tiles

                def emit_attn_pair(hp, es_tiles):
                    for j in range(2):
                        h = 2 * hp + j
                        psa = ps_a.tile([D + 1, S], F32, tag="attn", name="psa")
                        for kc in range(NK_CHUNKS):
                            nc.tensor.matmul(
                                psa[:],
                                v_sb[kc][:, h * (D + 1) : (h + 1) * (D + 1)],
                                es_tiles[kc][:, j * S : (j + 1) * S],
                                start=(kc == 0),
                                stop=(kc == NK_CHUNKS - 1),
                            )
                        dst = stg[h // 6][:, (h % 6) * S : (h % 6 + 1) * S]
                        if j == 0:
                            nc.vector.tensor_copy(dst, psa[:])
                        else:
                            nc.scalar.activation(
                                out=dst,
                                in_=psa[:],
                                func=mybir.ActivationFunctionType.Copy,
                            )

                def emit_norm_half(half):
                    sh = stg[half]
                    r12 = rp.tile([6, S], F16, tag="r12", name="r12")
                    nc.sync.dma_start(out=r12[:], in_=sh[D : D + 1, :])
                    r12f = rp.tile([6, S], F32, tag="r12f", name="r12f")
                    nc.vector.tensor_copy(r12f[:], r12[:])
                    rrf = rp.tile([6, S], F32, tag="rrf", name="rrf")
                    nc.vector.reciprocal_approx_fast(out=rrf[:], in_=r12f[:])
                    rr = rp.tile([6, S], F16, tag="rr", name="rr")
                    nc.scalar.activation(
                        out=rr[:],
                        in_=rrf[:],
                        func=mybir.ActivationFunctionType.Copy,
                    )
                    rbsrc = rp.tile([1, 6 * S], F16, tag="rbs", name="rbs")
                    nc.sync.dma_start(out=rbsrc[:], in_=rr[:])
                    rbh = rp.tile([D, 6 * S], F16, tag="rbh", name="rbh")
                    nc.gpsimd.partition_broadcast(rbh[:], rbsrc[:])
                    for jj in range(6):
                        h = 6 * half + jj
                        hp, po = h // 2, (h % 2) * D
                        nc.vector.tensor_mul(
                            cat_sb[hp][po : po + D, :],
                            sh[0:D, jj * S : (jj + 1) * S],
                            rbh[:, jj * S : (jj + 1) * S],
                        )

                def v_ready(hp):
                    need = [(kc, 0) for kc in range(NK_CHUNKS)]
                    if hp >= 3:
                        need += [(kc, 1) for kc in range(NK_CHUNKS)]
                    return all(k in v_sb for k in need)

                # spread the work thunks across the head pairs
                pend = None
                for hp in range(H // 2):
                    drain_until(
                        lambda: hp in q_sb and hp in k_sb and em_t[0] is not None
                    )
                    es_tiles = emit_scores_pair(hp)
                    if pend is not None:
                        drain_until(lambda: v_ready(pend[0]))
                        emit_attn_pair(pend[0], pend[1])
                        if pend[0] == 2:
                            emit_norm_half(0)
                    pend = (hp, es_tiles)
                    target = (hp + 1) * nwork // (H // 2)
                    while wi < target:
                        work[wi]()
                        wi += 1
                emit_attn_pair(pend[0], pend[1])
                emit_norm_half(1)
                while wi < nwork:
                    work[wi]()
                    wi += 1
                return cat_sb

            # ---- prologue: batch 0 -------------------------------------
            # DMA order: wq, x0, wk, mask0 so the q projection can start
            # after ~2MB and k/scores follow while wv/wo stream in.
            wq_t = load_w(wq_d, "wq")
            xt0 = load_x(0)
            wk_t = load_w(wk_d, "wk")
            mt0 = load_mask(0)
            wv_load, wo_load = [None], [None]

            def loadwv():
                wv_load[0] = load_w(wv_d, "wv")

            def loadwo():
                wo_load[0] = load_w(wo_d, "wo")

            em_cur = [None]
            q_cur, k_cur, v_cur = {}, {}, {}
            # emit first q/k groups so scores pair 0 can start early
            qk_group(wq_t, xt0, 0, "q", 0, q_cur)
            qk_group(wk_t, xt0, 6, "k", 0, k_cur)

            def qg(co):
                return lambda: qk_group(wq_t, xt0, 0, "q", co, q_cur)

            def kg(co):
                return lambda: qk_group(wk_t, xt0, 6, "k", co, k_cur)

            def vg(sc, half):
                return lambda: v_group(wv_load[0], xt0, sc, half, v_cur)

            work0 = [
                em_thunk(mt0, em_cur),
                qg(1), kg(1), qg(2), kg(2),
                loadwv,
                vg(0, 0), vg(1, 0), vg(2, 0), vg(3, 0),
                qg(3), kg(3), qg(4), kg(4),
                vg(0, 1), vg(1, 1), vg(2, 1), vg(3, 1),
                qg(5), kg(5),
                loadwo,
            ]

            prev_cat = None
            work = work0
            for b in range(NB):
                em_next = [None]
                q_next, k_next, v_next = {}, {}, {}
                if b + 1 < NB:
                    xt_next = load_x(b + 1)
                    mt_next = load_mask(b + 1)
                    work.append(em_thunk(mt_next, em_next))
                if prev_cat is not None:
                    work += oproj_thunks(wo_load, b - 1, prev_cat)
                if b + 1 < NB:
                    work += qkv_thunks(wv_load, xt_next, q_next, k_next, v_next)
                prev_cat = attention(b, q_cur, k_cur, v_cur, em_cur, work)
                em_cur, q_cur, k_cur, v_cur = em_next, q_next, k_next, v_next
                work = []

            # final oproj, ki-split: the ki 0-2 half only needs cat chunks
            # 0-2 (normed in half 0), so it executes during the last norm
            # chain; ki 3-5 closes the accumulation once half 1 lands.
            pso = []
            for _ in range(2):
                pt = ps_s.tile([128, 2 * S], F32, tag="spair", name="ps_fo")
                pso.append(pt[:, 0:S])
                pso.append(pt[:, S : 2 * S])
            for _ in range(2):
                pt = ps_proj.tile([128, S], F32, tag="proj", name="ps_fo2")
                pso.append(pt[:])
            wo_t = wo_load[0]
            for co in range(NC_CHUNKS):
                for ki in range(3):
                    nc.tensor.matmul(
                        pso[co],
                        wview(wo_t, ki)[:, co * 128 : (co + 1) * 128],
                        prev_cat[ki][:],
                        start=(ki == 0),
                        stop=False,
                    )
            for co in range(NC_CHUNKS):
                for ki in range(3, NC_CHUNKS):
                    nc.tensor.matmul(
                        pso[co],
                        wview(wo_t, ki)[:, co * 128 : (co + 1) * 128],
                        prev_cat[ki][:],
                        start=False,
                        stop=(ki == NC_CHUNKS - 1),
                    )
                ot = op.tile([128, S], F16, tag=f"o{co}", name="ot")
                if co % 2 == 0:
                    nc.vector.tensor_scalar_add(
                        ot[:], pso[co], bcol[:, 12 + co : 13 + co]
                    )
                else:
                    nc.scalar.activation(
                        out=ot[:],
                        in_=pso[co],
                        func=mybir.ActivationFunctionType.Identity,
                        bias=bcol[:, 12 + co : 13 + co],
                    )
                nc.sync.dma_start(
                    out=y_d.ap()[NB - 1, co * 128 : (co + 1) * 128, :], in_=ot[:]
                )

    nc.compile()
    return nc


def _get_compiled():
    global _COMPILED
    if _COMPILED is None:
        _COMPILED = _build()
    return _COMPILED


def _headmajor(wT):
    """Permute the output-channel axis of a transposed weight from the
    reference's head-minor order (c = d*H + h) to head-major (c' = h*D + d)."""
    return np.ascontiguousarray(
        wT.reshape(C, D, H).transpose(0, 2, 1).reshape(C, C)
    )


def _headmajor_b(bv):
    return np.ascontiguousarray(bv.reshape(D, H).T.reshape(C))


def prepare_in_maps(hidden_state, mask, Wq, bq, Wk, bk, Wv, bv, Wo, bo):
    x = np.asarray(hidden_state).reshape(B, C, S)
    m = np.asarray(mask).reshape(B, S, S)
    scale = np.float32(D**-0.5)

    wqt = np.ascontiguousarray(
        (_headmajor(np.asarray(Wq).T).astype(np.float32) * scale).astype(np.float16)
    )
    wkt = _headmajor(np.asarray(Wk).T)
    wvt = _headmajor(np.asarray(Wv).T)
    wot = np.ascontiguousarray(np.asarray(Wo).T)

    bq_s = (_headmajor_b(np.asarray(bq)).astype(np.float32) * scale).astype(
        np.float32
    )
    bk_p = np.asarray(bk).astype(np.float32)
    bk_p = _headmajor_b(bk_p)
    # fold bv through attention (softmax weights sum to 1) into bo:
    # bo' = bo + Wo @ bv_headmajor
    bv_hm = _headmajor_b(np.asarray(bv).astype(np.float32))
    bo_p = np.asarray(bo).astype(np.float32) + np.asarray(Wo).astype(
        np.float32
    ) @ bv_hm
    bcols = np.zeros((128, 19), dtype=np.float32)
    for j in range(NC_CHUNKS):
        bcols[:, j] = bq_s[j * 128 : (j + 1) * 128]
        bcols[:, 6 + j] = bk_p[j * 128 : (j + 1) * 128]
        bcols[:, 12 + j] = bo_p[j * 128 : (j + 1) * 128]
    bcols[:, 18] = EM_BIAS

    shared = {
        "wqt": wqt,
        "wkt": wkt,
        "wvt": wvt,
        "wot": wot,
        "bcols": np.ascontiguousarray(bcols),
    }
    in_maps = []
    for i in range(NCORES):
        sl = slice(i * NB, (i + 1) * NB)
        in_maps.append(
            dict(
                shared,
                x=np.ascontiguousarray(x[sl]),
                mask=np.ascontiguousarray(m[sl]),
            )
        )
    return in_maps


def kernel(**inputs):
    nc = _get_compiled()
    in_maps = prepare_in_maps(**inputs)
    res = run_bass_kernel_spmd(nc, in_maps, core_ids=list(range(NCORES)))
    y = np.concatenate([res.results[i]["y"] for i in range(NCORES)], axis=0)
    return y.reshape(B, C, 1, S)


# revision 24
# speedup vs baseline: 1.2148x; 1.2148x over previous
"""Trainium2 Bass kernel for nn_Attention_65455301591248.

Multi-head attention: B=32, C=768, H=12 heads, S=512, D=64.
  q/k/v = W{q,k,v} @ x + b   (1x1 conv == channel GEMM), head-minor channel
  scores[k,h,q] = (q.k)/sqrt(D) + mask[k,q];  softmax over k
  attn = w @ v; concat head-major; out = Wo @ attn + bo

Sharding: pure data parallel over batch - 4 batches per core x 8 cores,
no collectives.

Per-core kernel strategy (v2):
  - Host pre-transposes weights (lhsT layout) and permutes Q/K/V output
    channels head-major (c' = h*64+d).  Wq/bq pre-scaled by 1/8.
    bv folded into bo on host (softmax weights sum to 1, so +bv passes
    through attention unchanged): bo' = bo + Wo @ bv_hm.  Exact.
  - Scores for a head PAIR go into one [128,1024] PSUM tile (2 banks):
    even head -> cols 0:512 / rows 0:63 of the PE array, odd head ->
    cols 512:1024 / rows 64:127, explicit tile_position so the two
    K=64 matmuls run CONCURRENTLY (row tiling).
  - One exp per (pair,kc) over [128,1024] on ACT; one DVE multiply by
    em2 (mask exp, column-duplicated so the operand is contiguous).
    em is scaled by 2^-6 so fp16 reciprocals / weights stay in normal
    range; the scale cancels between numerator and denominator.
  - attn = V^T-slab @ w with a ones column per head: PSUM row 64
    accumulates the softmax denominator for free.  Eviction writes all
    heads of a half-batch into one [65,3072] tile, so ONE gather DMA
    collects the 6 denominator rows; reciprocal via
    reciprocal_approx_fast (fp32); one scatter DMA -> partition-0
    staging; gpsimd partition_broadcast + fp16 2x-mode DVE multiplies
    normalize into the head-major concat buffer.
  - Single large DMAs for weights / x / mask; prologue orders
    wq,wk,x,mask before wv,wo so score matmuls start early and the PE
    clock (HAM) stays warm.
"""

import numpy as np

try:
    import concourse.bass as bass  # noqa: F401
except ImportError:  # pragma: no cover
    import sys

    sys.path.insert(0, "/opt/trn_rl_repo")

import concourse.bass as bass
import concourse.tile as tile
from concourse import bacc, mybir
from concourse.bass_utils import run_bass_kernel_spmd

B, C, H, S, D = 32, 768, 12, 512, 64
NCORES = 8
NB = B // NCORES  # batches per core
F16 = mybir.dt.float16
F32 = mybir.dt.float32
NC_CHUNKS = C // 128  # 6
NK_CHUNKS = S // 128  # 4
VROW = H * (D + 1)  # 780: per-head 64 v columns + 1 ones column
EM_BIAS = -6.0 * float(np.log(2.0))  # exp(mask)*2^-6

_COMPILED = None


def _build():
    """Build + compile the per-core Bass program (runs on each of 8 cores)."""
    nc = bacc.Bacc("TRN2", target_bir_lowering=False, debug=False)

    x_d = nc.dram_tensor("x", [NB, C, S], F16, kind="ExternalInput")
    m_d = nc.dram_tensor("mask", [NB, S, S], F16, kind="ExternalInput")
    wq_d = nc.dram_tensor("wqt", [C, C], F16, kind="ExternalInput")
    wk_d = nc.dram_tensor("wkt", [C, C], F16, kind="ExternalInput")
    wv_d = nc.dram_tensor("wvt", [C, C], F16, kind="ExternalInput")
    wo_d = nc.dram_tensor("wot", [C, C], F16, kind="ExternalInput")
    # packed per-partition biases: cols 0-5 bq/8, 6-11 bk, 12-17 bo',
    # col 18 = EM_BIAS constant
    bcol_d = nc.dram_tensor("bcols", [128, 19], F32, kind="ExternalInput")
    y_d = nc.dram_tensor("y", [NB, C, S], F16, kind="ExternalOutput")

    with tile.TileContext(nc) as tc:
        with (
            tc.tile_pool(name="wpool", bufs=1) as wpool,
            tc.tile_pool(name="const", bufs=1) as const,
            tc.tile_pool(name="xp", bufs=2) as xp,
            tc.tile_pool(name="qk", bufs=2) as qk,
            tc.tile_pool(name="vp", bufs=2) as vp,
            tc.tile_pool(name="mp", bufs=2) as mp,
            tc.tile_pool(name="wexp", bufs=2) as wexp,
            tc.tile_pool(name="stgp", bufs=2) as stgp,
            tc.tile_pool(name="cat", bufs=2) as cat,
            tc.tile_pool(name="op", bufs=2) as op,
            tc.tile_pool(name="rp", bufs=2) as rp,
            tc.tile_pool(name="ps_proj", bufs=2, space="PSUM") as ps_proj,
            tc.tile_pool(name="ps_s", bufs=2, space="PSUM") as ps_s,
            tc.tile_pool(name="ps_a", bufs=2, space="PSUM") as ps_a,
        ):
            # ---- persistent weights / constants -------------------------
            def load_w(w_d, name):
                t = wpool.tile([128, NC_CHUNKS * C], F16, tag=name, name=name)
                nc.sync.dma_start(
                    out=t.rearrange("p (j c) -> p j c", c=C),
                    in_=w_d.ap().rearrange("(j p) c -> p j c", p=128),
                )
                return t

            bcol = const.tile([128, 19], F32, tag="bcol")
            nc.sync.dma_start(out=bcol[:], in_=bcol_d.ap()[:, :])

            def wview(t, ki):
                return t[:, ki * C : (ki + 1) * C]

            # ---- per-batch input loads ---------------------------------
            def load_x(b):
                t = xp.tile([128, NC_CHUNKS * S], F16, tag="x", name=f"x{b}")
                nc.sync.dma_start(
                    out=t.rearrange("p (j s) -> p j s", s=S),
                    in_=x_d.ap()[b].rearrange("(j p) s -> p j s", p=128),
                )
                return t

            def load_mask(b):
                t = mp.tile([128, NK_CHUNKS * S], F16, tag="mraw", name=f"m{b}")
                nc.sync.dma_start(
                    out=t.rearrange("p (kc q) -> p kc q", q=S),
                    in_=m_d.ap()[b].rearrange("(kc p) q -> p kc q", p=128),
                )
                return t

            def em_thunk(mt, em_out):
                # em2 = exp(mask)*2^-6, with each kc chunk duplicated so the
                # es multiply gets one contiguous [128,1024] operand per kc.
                def one():
                    e = mp.tile([128, 2 * NK_CHUNKS * S], F16, tag="em", name="em")
                    ev = e.rearrange("p (kc r q) -> p kc r q", r=2, q=S)
                    mv = mt.rearrange("p (kc q) -> p kc q", q=S)
                    for r in range(2):
                        nc.scalar.activation(
                            out=ev[:, :, r, :],
                            in_=mv[:],
                            func=mybir.ActivationFunctionType.Exp,
                            bias=bcol[:, 18:19],
                        )
                    em_out[0] = e

                return one

            # ---- projection groups -------------------------------------
            def qk_group(w_t, xt, bias_col, name, co, outs):
                ps = ps_proj.tile([128, S], F32, tag="proj", name="ps_p")
                for ki in range(NC_CHUNKS):
                    nc.tensor.matmul(
                        ps[:],
                        wview(w_t, ki)[:, co * 128 : (co + 1) * 128],
                        xt[:, ki * S : (ki + 1) * S],
                        start=(ki == 0),
                        stop=(ki == NC_CHUNKS - 1),
                    )
                dt = qk.tile([128, S], F16, tag=f"{name}{co}", name=f"{name}{co}")
                if co % 2 == 0:
                    nc.vector.tensor_scalar_add(
                        dt[:], ps[:], bcol[:, bias_col + co : bias_col + co + 1]
                    )
                else:
                    nc.scalar.activation(
                        out=dt[:],
                        in_=ps[:],
                        func=mybir.ActivationFunctionType.Identity,
                        bias=bcol[:, bias_col + co : bias_col + co + 1],
                    )
                outs[co] = dt

            def v_group(wv_t, xt, sc, half, v_out):
                # v^T projection chunk: out [s, c'] with per-head ones col
                if half == 0:
                    vt = vp.tile([128, VROW], F16, tag=f"v{sc}", name=f"v{sc}")
                    vv = vt.rearrange("p (h w) -> p h w", w=D + 1)
                    nc.vector.memset(vv[:, :, D : D + 1], 1.0)
                    v_out[sc] = vt
                else:
                    vt = v_out[sc]
                    vv = vt.rearrange("p (h w) -> p h w", w=D + 1)
                v_out[(sc, half)] = True
                hw = C // 2  # 384 = 6 heads
                ps = ps_proj.tile([128, hw], F32, tag="proj", name="ps_v")
                for ki in range(NC_CHUNKS):
                    nc.tensor.matmul(
                        ps[:],
                        xt[:, ki * S + sc * 128 : ki * S + (sc + 1) * 128],
                        wview(wv_t, ki)[:, half * hw : (half + 1) * hw],
                        start=(ki == 0),
                        stop=(ki == NC_CHUNKS - 1),
                    )
                if half == 0:
                    nc.scalar.activation(
                        out=vv[:, 0:6, 0:D],
                        in_=ps.rearrange("p (h w) -> p h w", w=D),
                        func=mybir.ActivationFunctionType.Copy,
                    )
                else:
                    nc.vector.tensor_copy(
                        vv[:, 6:12, 0:D],
                        ps.rearrange("p (h w) -> p h w", w=D),
                    )

            def qkv_thunks(wv_load, xt, q_out, k_out, v_out):
                th = []
                for co in range(NC_CHUNKS):
                    th.append(lambda co=co: qk_group(wq_t, xt, 0, "q", co, q_out))
                    th.append(lambda co=co: qk_group(wk_t, xt, 6, "k", co, k_out))
                for sc in range(NK_CHUNKS):
                    for half in range(2):
                        th.append(
                            lambda sc=sc, half=half: v_group(
                                wv_load[0], xt, sc, half, v_out
                            )
                        )
                return th

            def oproj_thunks(wo_load, b, cat_sb):
                def one(co):
                    ps = ps_proj.tile([128, S], F32, tag="proj", name="ps_o")
                    for ki in range(NC_CHUNKS):
                        nc.tensor.matmul(
                            ps[:],
                            wview(wo_load[0], ki)[:, co * 128 : (co + 1) * 128],
                            cat_sb[ki][:],
                            start=(ki == 0),
                            stop=(ki == NC_CHUNKS - 1),
                        )
                    ot = op.tile([128, S], F16, tag=f"o{co}", name="ot")
                    if co % 2 == 0:
                        nc.vector.tensor_scalar_add(
                            ot[:], ps[:], bcol[:, 12 + co : 13 + co]
                        )
                    else:
                        nc.scalar.activation(
                            out=ot[:],
                            in_=ps[:],
                            func=mybir.ActivationFunctionType.Identity,
                            bias=bcol[:, 12 + co : 13 + co],
                        )
                    nc.sync.dma_start(
                        out=y_d.ap()[b, co * 128 : (co + 1) * 128, :], in_=ot[:]
                    )

                return [lambda co=co: one(co) for co in range(NC_CHUNKS)]

            # ---- attention for one batch -------------------------------
            def attention(b, q_sb, k_sb, v_sb, em_t, work):
                # q_sb/k_sb/v_sb are dicts filled lazily by work thunks;
                # drain_until pulls work forward when an operand tile has
                # not been emitted yet (only matters for batch 0).
                cat_sb = []
                for j in range(NC_CHUNKS):
                    ct = cat.tile([128, S], F16, tag=f"c{j}", name=f"cat{j}")
                    cat_sb.append(ct)
                stg = [
                    stgp.tile([D + 1, 6 * S], F16, tag="stg0", name="stg0"),
                    stgp.tile([D + 1, 6 * S], F16, tag="stg1", name="stg1"),
                ]

                nwork = len(work)
                wi = 0

                def drain_until(cond):
                    nonlocal wi
                    while not cond():
                        assert wi < nwork, "work list exhausted before operand ready"
                        work[wi]()
                        wi += 1

                def emit_scores_pair(hp):
                    em = em_t[0]
                    es_tiles = []
                    for kc in range(NK_CHUNKS):
                        ps = ps_s.tile([128, 2 * S], F32, tag="spair", name="ps_sc")
                        for j in range(2):
                            po = j * D
                            nc.tensor.matmul(
                                ps[:, j * S : (j + 1) * S],
                                k_sb[hp][po : po + D, kc * 128 : (kc + 1) * 128],
                                q_sb[hp][po : po + D, :],
                                start=True,
                                stop=True,
                                tile_position=(po, 0),
                            )
                        es = wexp.tile([128, 2 * S], F16, tag=f"es{kc}", name="es")
                        nc.scalar.activation(
                            out=es[:],
                            in_=ps[:],
                            func=mybir.ActivationFunctionType.Exp,
                        )
                        nc.vector.tensor_mul(
                            es[:], es[:], em[:, kc * 2 * S : (kc + 1) * 2 * S]
                        )
                        es_tiles.append(es)
                    return es_tiles

                def emit_attn_pair(hp, es_tiles):
                    for j in range(2):
                        h = 2 * hp + j
                        psa = ps_a.tile([D + 1, S], F32, tag="attn", name="psa")
                        for kc in range(NK_CHUNKS):
                            nc.tensor.matmul(
                                psa[:],
                                v_sb[kc][:, h * (D + 1) : (h + 1) * (D + 1)],
                                es_tiles[kc][:, j * S : (j + 1) * S],
                                start=(kc == 0),
                                stop=(kc == NK_CHUNKS - 1),
                            )
                        dst = stg[h // 6][:, (h % 6) * S : (h % 6 + 1) * S]
                        if j == 0:
                            nc.vector.tensor_copy(dst, psa[:])
                        else:
                            nc.scalar.activation(
                                out=dst,
                                in_=psa[:],
                                func=mybir.ActivationFunctionType.Copy,
                            )

                def emit_norm_half(half):
                    sh = stg[half]
                    r12 = rp.tile([6, S], F16, tag="r12", name="r12")
                    nc.sync.dma_start(out=r12[:], in_=sh[D : D + 1, :])
                    r12f = rp.tile([6, S], F32, tag="r12f", name="r12f")
                    nc.vector.tensor_copy(r12f[:], r12[:])
                    rrf = rp.tile([6, S], F32, tag="rrf", name="rrf")
                    nc.vector.reciprocal_approx_fast(out=rrf[:], in_=r12f[:])
                    rr = rp.tile([6, S], F16, tag="rr", name="rr")
                    nc.scalar.activation(
                        out=rr[:],
                        in_=rrf[:],
                        func=mybir.ActivationFunctionType.Copy,
                    )
                    rbsrc = rp.tile([1, 6 * S], F16, tag="rbs", name="rbs")
                    nc.sync.dma_start(out=rbsrc[:], in_=rr[:])
                    for jj in range(6):
                        h = 6 * half + jj
                        hp, po = h // 2, (h % 2) * D
                        rb = rp.tile([D, S], F16, tag=f"rb{jj % 2}", name="rb")
                        nc.gpsimd.partition_broadcast(
                            rb[:], rbsrc[0:1, jj * S : (jj + 1) * S]
                        )
                        nc.vector.tensor_mul(
                            cat_sb[hp][po : po + D, :],
                            sh[0:D, jj * S : (jj + 1) * S],
                            rb[:],
                        )

                def v_ready(hp):
                    need = [(kc, 0) for kc in range(NK_CHUNKS)]
                    if hp >= 3:
                        need += [(kc, 1) for kc in range(NK_CHUNKS)]
                    return all(k in v_sb for k in need)

                # spread the work thunks across the head pairs
                pend = None
                for hp in range(H // 2):
                    drain_until(
                        lambda: hp in q_sb and hp in k_sb and em_t[0] is not None
                    )
                    es_tiles = emit_scores_pair(hp)
                    if pend is not None:
                        drain_until(lambda: v_ready(pend[0]))
                        emit_attn_pair(pend[0], pend[1])
                        if pend[0] == 2:
                            emit_norm_half(0)
                    pend = (hp, es_tiles)
                    target = (hp + 1) * nwork // (H // 2)
                    while wi < target:
                        work[wi]()
                        wi += 1
                emit_attn_pair(pend[0], pend[1])
                emit_norm_half(1)
                while wi < nwork:
                    work[wi]()
                    wi += 1
                return cat_sb

            # ---- prologue: batch 0 -------------------------------------
            # DMA order: wq, x0, wk, mask0 so the q projection can start
            # after ~2MB and k/scores follow while wv/wo stream in.
            wq_t = load_w(wq_d, "wq")
            xt0 = load_x(0)
            wk_t = load_w(wk_d, "wk")
            mt0 = load_mask(0)
            wv_load, wo_load = [None], [None]

            def loadwv():
                wv_load[0] = load_w(wv_d, "wv")

            def loadwo():
                wo_load[0] = load_w(wo_d, "wo")

            em_cur = [None]
            q_cur, k_cur, v_cur = {}, {}, {}
            # emit first q/k groups so scores pair 0 can start early
            qk_group(wq_t, xt0, 0, "q", 0, q_cur)
            qk_group(wk_t, xt0, 6, "k", 0, k_cur)

            def qg(co):
                return lambda: qk_group(wq_t, xt0, 0, "q", co, q_cur)

            def kg(co):
                return lambda: qk_group(wk_t, xt0, 6, "k", co, k_cur)

            def vg(sc, half):
                return lambda: v_group(wv_load[0], xt0, sc, half, v_cur)

            work0 = [
                em_thunk(mt0, em_cur),
                qg(1), kg(1), qg(2), kg(2),
                loadwv,
                vg(0, 0), vg(1, 0), vg(2, 0), vg(3, 0),
                qg(3), kg(3), qg(4), kg(4),
                vg(0, 1), vg(1, 1), vg(2, 1), vg(3, 1),
                qg(5), kg(5),
                loadwo,
            ]

            prev_cat = None
            work = work0
            for b in range(NB):
                em_next = [None]
                q_next, k_next, v_next = {}, {}, {}
                if b + 1 < NB:
                    xt_next = load_x(b + 1)
                    mt_next = load_mask(b + 1)
                    work.append(em_thunk(mt_next, em_next))
                if prev_cat is not None:
                    work += oproj_thunks(wo_load, b - 1, prev_cat)
                if b + 1 < NB:
                    work += qkv_thunks(wv_load, xt_next, q_next, k_next, v_next)
                prev_cat = attention(b, q_cur, k_cur, v_cur, em_cur, work)
                em_cur, q_cur, k_cur, v_cur = em_next, q_next, k_next, v_next
                work = []

            # final oproj, ki-split: the ki 0-2 half only needs cat chunks
            # 0-2 (normed in half 0), so it executes during the last norm
            # chain; ki 3-5 closes the accumulation once half 1 lands.
            pso = []
            for _ in range(2):
                pt = ps_s.tile([128, 2 * S], F32, tag="spair", name="ps_fo")
                pso.append(pt[:, 0:S])
                pso.append(pt[:, S : 2 * S])
            for _ in range(2):
                pt = ps_proj.tile([128, S], F32, tag="proj", name="ps_fo2")
                pso.append(pt[:])
            wo_t = wo_load[0]
            for co in range(NC_CHUNKS):
                for ki in range(3):
                    nc.tensor.matmul(
                        pso[co],
                        wview(wo_t, ki)[:, co * 128 : (co + 1) * 128],
                        prev_cat[ki][:],
                        start=(ki == 0),
                        stop=False,
                    )
            for co in range(NC_CHUNKS):
                for ki in range(3, NC_CHUNKS):
                    nc.tensor.matmul(
                        pso[co],
                        wview(wo_t, ki)[:, co * 128 : (co + 1) * 128],
                        prev_cat[ki][:],
                        start=False,
                        stop=(ki == NC_CHUNKS - 1),
                    )
                ot = op.tile([128, S], F16, tag=f"o{co}", name="ot")
                if co % 2 == 0:
                    nc.vector.tensor_scalar_add(
                        ot[:], pso[co], bcol[:, 12 + co : 13 + co]
                    )
                else:
                    nc.scalar.activation(
                        out=ot[:],
                        in_=pso[co],
                        func=mybir.ActivationFunctionType.Identity,
                        bias=bcol[:, 12 + co : 13 + co],
                    )
                nc.sync.dma_start(
                    out=y_d.ap()[NB - 1, co * 128 : (co + 1) * 128, :], in_=ot[:]
                )

    nc.compile()
    return nc


def _get_compiled():
    global _COMPILED
    if _COMPILED is None:
        _COMPILED = _build()
    return _COMPILED


def _headmajor(wT):
    """Permute the output-channel axis of a transposed weight from the
    reference's head-minor order (c = d*H + h) to head-major (c' = h*D + d)."""
    return np.ascontiguousarray(
        wT.reshape(C, D, H).transpose(0, 2, 1).reshape(C, C)
    )


def _headmajor_b(bv):
    return np.ascontiguousarray(bv.reshape(D, H).T.reshape(C))


def prepare_in_maps(hidden_state, mask, Wq, bq, Wk, bk, Wv, bv, Wo, bo):
    x = np.asarray(hidden_state).reshape(B, C, S)
    m = np.asarray(mask).reshape(B, S, S)
    scale = np.float32(D**-0.5)

    wqt = np.ascontiguousarray(
        (_headmajor(np.asarray(Wq).T).astype(np.float32) * scale).astype(np.float16)
    )
    wkt = _headmajor(np.asarray(Wk).T)
    wvt = _headmajor(np.asarray(Wv).T)
    wot = np.ascontiguousarray(np.asarray(Wo).T)

    bq_s = (_headmajor_b(np.asarray(bq)).astype(np.float32) * scale).astype(
        np.float32
    )
    bk_p = np.asarray(bk).astype(np.float32)
    bk_p = _headmajor_b(bk_p)
    # fold bv through attention (softmax weights sum to 1) into bo:
    # bo' = bo + Wo @ bv_headmajor
    bv_hm = _headmajor_b(np.asarray(bv).astype(np.float32))
    bo_p = np.asarray(bo).astype(np.float32) + np.asarray(Wo).astype(
        np.float32
    ) @ bv_hm
    bcols = np.zeros((128, 19), dtype=np.float32)
    for j in range(NC_CHUNKS):
        bcols[:, j] = bq_s[j * 128 : (j + 1) * 128]
        bcols[:, 6 + j] = bk_p[j * 128 : (j + 1) * 128]
        bcols[:, 12 + j] = bo_p[j * 128 : (j + 1) * 128]
    bcols[:, 18] = EM_BIAS

    shared = {
        "wqt": wqt,
        "wkt": wkt,
        "wvt": wvt,
        "wot": wot,
        "bcols": np.ascontiguousarray(bcols),
    }
    in_maps = []
    for i in range(NCORES):
        sl = slice(i * NB, (i + 1) * NB)
        in_maps.append(
            dict(
                shared,
                x=np.ascontiguousarray(x[sl]),
                mask=np.ascontiguousarray(m[sl]),
            )
        )
    return in_maps


def kernel(**inputs):
    nc = _get_compiled()
    in_maps = prepare_in_maps(**inputs)
    res = run_bass_kernel_spmd(nc, in_maps, core_ids=list(range(NCORES)))
    y = np.concatenate([res.results[i]["y"] for i in range(NCORES)], axis=0)
    return y.reshape(B, C, 1, S)


# revision 32
# speedup vs baseline: 1.2349x; 1.0165x over previous
"""Trainium2 Bass kernel for nn_Attention_65455301591248.

Multi-head attention: B=32, C=768, H=12 heads, S=512, D=64.
  q/k/v = W{q,k,v} @ x + b   (1x1 conv == channel GEMM), head-minor channel
  scores[k,h,q] = (q.k)/sqrt(D) + mask[k,q];  softmax over k
  attn = w @ v; concat head-major; out = Wo @ attn + bo

Sharding: pure data parallel over batch - 4 batches per core x 8 cores,
no collectives.

Per-core kernel strategy (v2):
  - Host pre-transposes weights (lhsT layout) and permutes Q/K/V output
    channels head-major (c' = h*64+d).  Wq/bq pre-scaled by 1/8.
    bv folded into bo on host (softmax weights sum to 1, so +bv passes
    through attention unchanged): bo' = bo + Wo @ bv_hm.  Exact.
  - Scores for a head PAIR go into one [128,1024] PSUM tile (2 banks):
    even head -> cols 0:512 / rows 0:63 of the PE array, odd head ->
    cols 512:1024 / rows 64:127, explicit tile_position so the two
    K=64 matmuls run CONCURRENTLY (row tiling).
  - One exp per (pair,kc) over [128,1024] on ACT; one DVE multiply by
    em2 (mask exp, column-duplicated so the operand is contiguous).
    em is scaled by 2^-6 so fp16 reciprocals / weights stay in normal
    range; the scale cancels between numerator and denominator.
  - attn = V^T-slab @ w with a ones column per head: PSUM row 64
    accumulates the softmax denominator for free.  Eviction writes all
    heads of a half-batch into one [65,3072] tile, so ONE gather DMA
    collects the 6 denominator rows; reciprocal via
    reciprocal_approx_fast (fp32); one scatter DMA -> partition-0
    staging; gpsimd partition_broadcast + fp16 2x-mode DVE multiplies
    normalize into the head-major concat buffer.
  - Single large DMAs for weights / x / mask; prologue orders
    wq,wk,x,mask before wv,wo so score matmuls start early and the PE
    clock (HAM) stays warm.
"""

import numpy as np

try:
    import concourse.bass as bass  # noqa: F401
except ImportError:  # pragma: no cover
    import sys

    sys.path.insert(0, "/opt/trn_rl_repo")

import concourse.bass as bass
import concourse.tile as tile
from concourse import bacc, mybir
from concourse.bass_utils import run_bass_kernel_spmd

B, C, H, S, D = 32, 768, 12, 512, 64
NCORES = 8
NB = B // NCORES  # batches per core
F16 = mybir.dt.float16
F32 = mybir.dt.float32
NC_CHUNKS = C // 128  # 6
NK_CHUNKS = S // 128  # 4
VROW = H * (D + 1)  # 780: per-head 64 v columns + 1 ones column
EM_BIAS = -6.0 * float(np.log(2.0))  # exp(mask)*2^-6

_COMPILED = None


def _build():
    """Build + compile the per-core Bass program (runs on each of 8 cores)."""
    nc = bacc.Bacc("TRN2", target_bir_lowering=False, debug=False)

    x_d = nc.dram_tensor("x", [NB, C, S], F16, kind="ExternalInput")
    m_d = nc.dram_tensor("mask", [NB, S, S], F16, kind="ExternalInput")
    wq_d = nc.dram_tensor("wqt", [C, C], F16, kind="ExternalInput")
    wk_d = nc.dram_tensor("wkt", [C, C], F16, kind="ExternalInput")
    wv_d = nc.dram_tensor("wvt", [C, C], F16, kind="ExternalInput")
    wo_d = nc.dram_tensor("wot", [C, C], F16, kind="ExternalInput")
    # packed per-partition biases: cols 0-5 bq/8, 6-11 bk, 12-17 bo',
    # col 18 = EM_BIAS constant
    bcol_d = nc.dram_tensor("bcols", [128, 19], F32, kind="ExternalInput")
    y_d = nc.dram_tensor("y", [NB, C, S], F16, kind="ExternalOutput")

    with tile.TileContext(nc) as tc:
        with (
            tc.tile_pool(name="wpool", bufs=1) as wpool,
            tc.tile_pool(name="const", bufs=1) as const,
            tc.tile_pool(name="xp", bufs=2) as xp,
            tc.tile_pool(name="qk", bufs=2) as qk,
            tc.tile_pool(name="vp", bufs=2) as vp,
            tc.tile_pool(name="mp", bufs=2) as mp,
            tc.tile_pool(name="wexp", bufs=2) as wexp,
            tc.tile_pool(name="stgp", bufs=2) as stgp,
            tc.tile_pool(name="cat", bufs=2) as cat,
            tc.tile_pool(name="op", bufs=2) as op,
            tc.tile_pool(name="rp", bufs=2) as rp,
            tc.tile_pool(name="ps_proj", bufs=2, space="PSUM") as ps_proj,
            tc.tile_pool(name="ps_s", bufs=2, space="PSUM") as ps_s,
            tc.tile_pool(name="ps_a", bufs=2, space="PSUM") as ps_a,
        ):
            # ---- persistent weights / constants -------------------------
            def load_w(w_d, name, chunked=False):
                t = wpool.tile([128, NC_CHUNKS * C], F16, tag=name, name=name)
                if chunked:
                    # per-chunk DMAs give fine-grained deps: the ki-th
                    # accumulation matmul only waits for chunk ki
                    for j in range(NC_CHUNKS):
                        nc.sync.dma_start(
                            out=t[:, j * C : (j + 1) * C],
                            in_=w_d.ap()[j * 128 : (j + 1) * 128, :],
                        )
                else:
                    nc.sync.dma_start(
                        out=t.rearrange("p (j c) -> p j c", c=C),
                        in_=w_d.ap().rearrange("(j p) c -> p j c", p=128),
                    )
                return t

            bcol = const.tile([128, 19], F32, tag="bcol")
            nc.sync.dma_start(out=bcol[:], in_=bcol_d.ap()[:, :])

            def wview(t, ki):
                return t[:, ki * C : (ki + 1) * C]

            # ---- per-batch input loads ---------------------------------
            def load_x(b, chunked=False):
                t = xp.tile([128, NC_CHUNKS * S], F16, tag="x", name=f"x{b}")
                if chunked:
                    for j in range(NC_CHUNKS):
                        nc.sync.dma_start(
                            out=t[:, j * S : (j + 1) * S],
                            in_=x_d.ap()[b, j * 128 : (j + 1) * 128, :],
                        )
                else:
                    nc.sync.dma_start(
                        out=t.rearrange("p (j s) -> p j s", s=S),
                        in_=x_d.ap()[b].rearrange("(j p) s -> p j s", p=128),
                    )
                return t

            def load_mask(b):
                t = mp.tile([128, NK_CHUNKS * S], F16, tag="mraw", name=f"m{b}")
                nc.sync.dma_start(
                    out=t.rearrange("p (kc q) -> p kc q", q=S),
                    in_=m_d.ap()[b].rearrange("(kc p) q -> p kc q", p=128),
                )
                return t

            def em_thunk(mt, em_out):
                # em2 = exp(mask)*2^-6, with each kc chunk duplicated so the
                # es multiply gets one contiguous [128,1024] operand per kc.
                def one():
                    e = mp.tile([128, 2 * NK_CHUNKS * S], F16, tag="em", name="em")
                    ev = e.rearrange("p (kc r q) -> p kc r q", r=2, q=S)
                    mv = mt.rearrange("p (kc q) -> p kc q", q=S)
                    for r in range(2):
                        nc.scalar.activation(
                            out=ev[:, :, r, :],
                            in_=mv[:],
                            func=mybir.ActivationFunctionType.Exp,
                            bias=bcol[:, 18:19],
                        )
                    em_out[0] = e

                return one

            # ---- projection groups -------------------------------------
            def qk_group(w_t, xt, bias_col, name, co, outs):
                ps = ps_proj.tile([128, S], F32, tag="proj", name="ps_p")
                for ki in range(NC_CHUNKS):
                    nc.tensor.matmul(
                        ps[:],
                        wview(w_t, ki)[:, co * 128 : (co + 1) * 128],
                        xt[:, ki * S : (ki + 1) * S],
                        start=(ki == 0),
                        stop=(ki == NC_CHUNKS - 1),
                    )
                dt = qk.tile([128, S], F16, tag=f"{name}{co}", name=f"{name}{co}")
                if co % 2 == 0:
                    nc.vector.tensor_scalar_add(
                        dt[:], ps[:], bcol[:, bias_col + co : bias_col + co + 1]
                    )
                else:
                    nc.scalar.activation(
                        out=dt[:],
                        in_=ps[:],
                        func=mybir.ActivationFunctionType.Identity,
                        bias=bcol[:, bias_col + co : bias_col + co + 1],
                    )
                outs[co] = dt

            def v_group(wv_t, xt, sc, half, v_out):
                # v^T projection chunk: out [s, c'] with per-head ones col
                if half == 0:
                    vt = vp.tile([128, VROW], F16, tag=f"v{sc}", name=f"v{sc}")
                    vv = vt.rearrange("p (h w) -> p h w", w=D + 1)
                    nc.vector.memset(vv[:, :, D : D + 1], 1.0)
                    v_out[sc] = vt
                else:
                    vt = v_out[sc]
                    vv = vt.rearrange("p (h w) -> p h w", w=D + 1)
                v_out[(sc, half)] = True
                hw = C // 2  # 384 = 6 heads
                ps = ps_proj.tile([128, hw], F32, tag="proj", name="ps_v")
                for ki in range(NC_CHUNKS):
                    nc.tensor.matmul(
                        ps[:],
                        xt[:, ki * S + sc * 128 : ki * S + (sc + 1) * 128],
                        wview(wv_t, ki)[:, half * hw : (half + 1) * hw],
                        start=(ki == 0),
                        stop=(ki == NC_CHUNKS - 1),
                    )
                if half == 0:
                    nc.scalar.activation(
                        out=vv[:, 0:6, 0:D],
                        in_=ps.rearrange("p (h w) -> p h w", w=D),
                        func=mybir.ActivationFunctionType.Copy,
                    )
                else:
                    nc.vector.tensor_copy(
                        vv[:, 6:12, 0:D],
                        ps.rearrange("p (h w) -> p h w", w=D),
                    )

            def qkv_thunks(wv_load, xt, q_out, k_out, v_out):
                th = []
                for co in range(NC_CHUNKS):
                    th.append(lambda co=co: qk_group(wq_t, xt, 0, "q", co, q_out))
                    th.append(lambda co=co: qk_group(wk_t, xt, 6, "k", co, k_out))
                for sc in range(NK_CHUNKS):
                    for half in range(2):
                        th.append(
                            lambda sc=sc, half=half: v_group(
                                wv_load[0], xt, sc, half, v_out
                            )
                        )
                return th

            def oproj_thunks(wo_load, b, cat_sb):
                def one(co):
                    ps = ps_proj.tile([128, S], F32, tag="proj", name="ps_o")
                    for ki in range(NC_CHUNKS):
                        nc.tensor.matmul(
                            ps[:],
                            wview(wo_load[0], ki)[:, co * 128 : (co + 1) * 128],
                            cat_sb[ki][:],
                            start=(ki == 0),
                            stop=(ki == NC_CHUNKS - 1),
                        )
                    ot = op.tile([128, S], F16, tag=f"o{co}", name="ot")
                    if co % 2 == 0:
                        nc.vector.tensor_scalar_add(
                            ot[:], ps[:], bcol[:, 12 + co : 13 + co]
                        )
                    else:
                        nc.scalar.activation(
                            out=ot[:],
                            in_=ps[:],
                            func=mybir.ActivationFunctionType.Identity,
                            bias=bcol[:, 12 + co : 13 + co],
                        )
                    nc.sync.dma_start(
                        out=y_d.ap()[b, co * 128 : (co + 1) * 128, :], in_=ot[:]
                    )

                return [lambda co=co: one(co) for co in range(NC_CHUNKS)]

            # ---- attention for one batch -------------------------------
            def attention(b, q_sb, k_sb, v_sb, em_t, work, last=False):
                # q_sb/k_sb/v_sb are dicts filled lazily by work thunks;
                # drain_until pulls work forward when an operand tile has
                # not been emitted yet (only matters for batch 0).
                cat_sb = []
                for j in range(NC_CHUNKS):
                    ct = cat.tile([128, S], F16, tag=f"c{j}", name=f"cat{j}")
                    cat_sb.append(ct)
                stg = [
                    stgp.tile([D + 1, 6 * S], F16, tag="stg0", name="stg0"),
                    stgp.tile([D + 1, 6 * S], F16, tag="stg1", name="stg1"),
                ]

                nwork = len(work)
                wi = 0

                def drain_until(cond):
                    nonlocal wi
                    while not cond():
                        assert wi < nwork, "work list exhausted before operand ready"
                        work[wi]()
                        wi += 1

                def emit_scores_pair(hp):
                    em = em_t[0]
                    es_tiles = []
                    for kc in range(NK_CHUNKS):
                        ps = ps_s.tile([128, 2 * S], F32, tag="spair", name="ps_sc")
                        for j in range(2):
                            po = j * D
                            nc.tensor.matmul(
                                ps[:, j * S : (j + 1) * S],
                                k_sb[hp][po : po + D, kc * 128 : (kc + 1) * 128],
                                q_sb[hp][po : po + D, :],
                                start=True,
                                stop=True,
                                tile_position=(po, 0),
                            )
                        es = wexp.tile([128, 2 * S], F16, tag=f"es{kc}", name="es")
                        nc.scalar.activation(
                            out=es[:],
                            in_=ps[:],
                            func=mybir.ActivationFunctionType.Exp,
                        )
                        nc.vector.tensor_mul(
                            es[:], es[:], em[:, kc * 2 * S : (kc + 1) * 2 * S]
                        )
                        es_tiles.append(es)
                    return es_tiles

                def emit_attn_pair(hp, es_tiles):
                    for j in range(2):
                        h = 2 * hp + j
                        psa = ps_a.tile([D + 1, S], F32, tag="attn", name="psa")
                        for kc in range(NK_CHUNKS):
                            nc.tensor.matmul(
                                psa[:],
                                v_sb[kc][:, h * (D + 1) : (h + 1) * (D + 1)],
                                es_tiles[kc][:, j * S : (j + 1) * S],
                                start=(kc == 0),
                                stop=(kc == NK_CHUNKS - 1),
                            )
                        dst = stg[h // 6][:, (h % 6) * S : (h % 6 + 1) * S]
                        if j == 0:
                            nc.vector.tensor_copy(dst, psa[:])
                        else:
                            nc.scalar.activation(
                                out=dst,
                                in_=psa[:],
                                func=mybir.ActivationFunctionType.Copy,
                            )

                def emit_norm_half(half):
                    sh = stg[half]
                    r12 = rp.tile([6, S], F16, tag="r12", name="r12")
                    nc.sync.dma_start(out=r12[:], in_=sh[D : D + 1, :])
                    r12f = rp.tile([6, S], F32, tag="r12f", name="r12f")
                    nc.vector.tensor_copy(r12f[:], r12[:])
                    rrf = rp.tile([6, S], F32, tag="rrf", name="rrf")
                    nc.vector.reciprocal_approx_fast(out=rrf[:], in_=r12f[:])
                    rr = rp.tile([6, S], F16, tag="rr", name="rr")
                    nc.scalar.activation(
                        out=rr[:],
                        in_=rrf[:],
                        func=mybir.ActivationFunctionType.Copy,
                    )
                    rbsrc = rp.tile([1, 6 * S], F16, tag="rbs", name="rbs")
                    nc.sync.dma_start(out=rbsrc[:], in_=rr[:])
                    for jj in range(6):
                        h = 6 * half + jj
                        hp, po = h // 2, (h % 2) * D
                        rb = rp.tile([D, S], F16, tag=f"rb{jj % 2}", name="rb")
                        nc.gpsimd.partition_broadcast(
                            rb[:], rbsrc[0:1, jj * S : (jj + 1) * S]
                        )
                        nc.vector.tensor_mul(
                            cat_sb[hp][po : po + D, :],
                            sh[0:D, jj * S : (jj + 1) * S],
                            rb[:],
                        )

                def emit_norm_pair(hp):
                    # fine-grained tail norm (last batch, pairs 3-5): one
                    # short chain per pair so the final oproj isn't gated on
                    # a 6-head chain
                    sh = stg[1]
                    c0 = (2 * hp - 6) * S
                    r2 = rp.tile([6, S], F16, tag="r12", name="r2")[0:2]
                    nc.sync.dma_start(out=r2, in_=sh[D : D + 1, c0 : c0 + 2 * S])
                    r2f = rp.tile([6, S], F32, tag="r12f", name="r2f")[0:2]
                    nc.vector.tensor_copy(r2f, r2)
                    rr2f = rp.tile([6, S], F32, tag="rrf", name="rr2f")[0:2]
                    nc.vector.reciprocal_approx_fast(out=rr2f, in_=r2f)
                    rr2 = rp.tile([6, S], F16, tag="rr", name="rr2")[0:2]
                    nc.scalar.activation(
                        out=rr2,
                        in_=rr2f,
                        func=mybir.ActivationFunctionType.Copy,
                    )
                    rbsrc2 = rp.tile([1, 6 * S], F16, tag="rbs", name="rbs2")[
                        :, 0 : 2 * S
                    ]
                    nc.sync.dma_start(out=rbsrc2, in_=rr2)
                    for j in range(2):
                        rb = rp.tile([D, S], F16, tag=f"rb{j}", name="rb2")
                        nc.gpsimd.partition_broadcast(
                            rb[:], rbsrc2[0:1, j * S : (j + 1) * S]
                        )
                        nc.vector.tensor_mul(
                            cat_sb[hp][j * D : (j + 1) * D, :],
                            sh[0:D, c0 + j * S : c0 + (j + 1) * S],
                            rb[:],
                        )

                def v_ready(hp):
                    need = [(kc, 0) for kc in range(NK_CHUNKS)]
                    if hp >= 3:
                        need += [(kc, 1) for kc in range(NK_CHUNKS)]
                    return all(k in v_sb for k in need)

                # spread the work thunks across the head pairs
                pend = None
                for hp in range(H // 2):
                    drain_until(
                        lambda: hp in q_sb and hp in k_sb and em_t[0] is not None
                    )
                    es_tiles = emit_scores_pair(hp)
                    if pend is not None:
                        drain_until(lambda: v_ready(pend[0]))
                        emit_attn_pair(pend[0], pend[1])
                        if pend[0] == 2:
                            emit_norm_half(0)
                        elif last and pend[0] >= 3:
                            emit_norm_pair(pend[0])
                    pend = (hp, es_tiles)
                    target = (hp + 1) * nwork // (H // 2)
                    while wi < target:
                        work[wi]()
                        wi += 1
                emit_attn_pair(pend[0], pend[1])
                if last:
                    emit_norm_pair(5)
                else:
                    emit_norm_half(1)
                while wi < nwork:
                    work[wi]()
                    wi += 1
                return cat_sb

            # ---- prologue: batch 0 -------------------------------------
            # Dummy matmuls on a zeroed tile bridge the DMA-startup window
            # so the PE clock (HAM) is already warm when real work lands.
            dummy = const.tile([128, S], F16, tag="dummy")
            nc.vector.memset(dummy[:], 0.0)
            for i in range(36):
                psd = ps_proj.tile([128, S], F32, tag="proj", name="ps_d")
                nc.tensor.matmul(
                    psd[:], dummy[:, 0:128], dummy[:], start=True, stop=True
                )
                if i == 35:
                    nc.vector.tensor_copy(dummy[:, 0:1], psd[:, 0:1])

            # DMA order: wq/x interleaved per chunk (matmul ki only waits
            # for chunk ki), then wk, mask0, wv, wo.
            wq_t = wpool.tile([128, NC_CHUNKS * C], F16, tag="wq", name="wq")
            xt0 = xp.tile([128, NC_CHUNKS * S], F16, tag="x", name="x0")
            for j in range(NC_CHUNKS):
                nc.sync.dma_start(
                    out=wq_t[:, j * C : (j + 1) * C],
                    in_=wq_d.ap()[j * 128 : (j + 1) * 128, :],
                )
                nc.sync.dma_start(
                    out=xt0[:, j * S : (j + 1) * S],
                    in_=x_d.ap()[0, j * 128 : (j + 1) * 128, :],
                )
            wk_t = load_w(wk_d, "wk", chunked=True)
            mt0 = load_mask(0)
            wv_load, wo_load = [None], [None]

            def loadwv():
                wv_load[0] = load_w(wv_d, "wv")

            def loadwo():
                wo_load[0] = load_w(wo_d, "wo")

            em_cur = [None]
            q_cur, k_cur, v_cur = {}, {}, {}
            # emit first q/k groups so scores pair 0 can start early
            qk_group(wq_t, xt0, 0, "q", 0, q_cur)
            qk_group(wk_t, xt0, 6, "k", 0, k_cur)

            def qg(co):
                return lambda: qk_group(wq_t, xt0, 0, "q", co, q_cur)

            def kg(co):
                return lambda: qk_group(wk_t, xt0, 6, "k", co, k_cur)

            def vg(sc, half):
                return lambda: v_group(wv_load[0], xt0, sc, half, v_cur)

            work0 = [
                em_thunk(mt0, em_cur),
                qg(1), kg(1), qg(2), kg(2),
                loadwv,
                vg(0, 0), vg(1, 0), vg(2, 0), vg(3, 0),
                qg(3), kg(3), qg(4), kg(4),
                vg(0, 1), vg(1, 1), vg(2, 1), vg(3, 1),
                qg(5), kg(5),
                loadwo,
            ]

            prev_cat = None
            work = work0
            for b in range(NB):
                em_next = [None]
                q_next, k_next, v_next = {}, {}, {}
                if b + 1 < NB:
                    xt_next = load_x(b + 1)
                    mt_next = load_mask(b + 1)
                    work.append(em_thunk(mt_next, em_next))
                if prev_cat is not None:
                    work += oproj_thunks(wo_load, b - 1, prev_cat)
                if b + 1 < NB:
                    work += qkv_thunks(wv_load, xt_next, q_next, k_next, v_next)
                prev_cat = attention(
                    b, q_cur, k_cur, v_cur, em_cur, work, last=(b == NB - 1)
                )
                em_cur, q_cur, k_cur, v_cur = em_next, q_next, k_next, v_next
                work = []

            # final oproj, ki-split: chunks 0-4 are normed (half 0 + pair
            # norms 3,4) well before the pair-5 chain lands, so the first
            # 30 matmuls execute during that chain; ki=5 closes each group.
            pso = []
            for _ in range(2):
                pt = ps_s.tile([128, 2 * S], F32, tag="spair", name="ps_fo")
                pso.append(pt[:, 0:S])
                pso.append(pt[:, S : 2 * S])
            for _ in range(2):
                pt = ps_proj.tile([128, S], F32, tag="proj", name="ps_fo2")
                pso.append(pt[:])
            wo_t = wo_load[0]
            for co in range(NC_CHUNKS):
                for ki in range(5):
                    nc.tensor.matmul(
                        pso[co],
                        wview(wo_t, ki)[:, co * 128 : (co + 1) * 128],
                        prev_cat[ki][:],
                        start=(ki == 0),
                        stop=False,
                    )
            for co in range(NC_CHUNKS):
                for ki in range(5, NC_CHUNKS):
                    nc.tensor.matmul(
                        pso[co],
                        wview(wo_t, ki)[:, co * 128 : (co + 1) * 128],
                        prev_cat[ki][:],
                        start=False,
                        stop=(ki == NC_CHUNKS - 1),
                    )
                ot = op.tile([128, S], F16, tag=f"o{co}", name="ot")
                if co % 2 == 0:
                    nc.vector.tensor_scalar_add(
                        ot[:], pso[co], bcol[:, 12 + co : 13 + co]
                    )
                else:
                    nc.scalar.activation(
                        out=ot[:],
                        in_=pso[co],
                        func=mybir.ActivationFunctionType.Identity,
                        bias=bcol[:, 12 + co : 13 + co],
                    )
                nc.sync.dma_start(
                    out=y_d.ap()[NB - 1, co * 128 : (co + 1) * 128, :], in_=ot[:]
                )

    nc.compile()
    return nc


def _get_compiled():
    global _COMPILED
    if _COMPILED is None:
        _COMPILED = _build()
    return _COMPILED


def _headmajor(wT):
    """Permute the output-channel axis of a transposed weight from the
    reference's head-minor order (c = d*H + h) to head-major (c' = h*D + d)."""
    return np.ascontiguousarray(
        wT.reshape(C, D, H).transpose(0, 2, 1).reshape(C, C)
    )


def _headmajor_b(bv):
    return np.ascontiguousarray(bv.reshape(D, H).T.reshape(C))


def prepare_in_maps(hidden_state, mask, Wq, bq, Wk, bk, Wv, bv, Wo, bo):
    x = np.asarray(hidden_state).reshape(B, C, S)
    m = np.asarray(mask).reshape(B, S, S)
    scale = np.float32(D**-0.5)

    wqt = np.ascontiguousarray(
        (_headmajor(np.asarray(Wq).T).astype(np.float32) * scale).astype(np.float16)
    )
    wkt = _headmajor(np.asarray(Wk).T)
    wvt = _headmajor(np.asarray(Wv).T)
    wot = np.ascontiguousarray(np.asarray(Wo).T)

    bq_s = (_headmajor_b(np.asarray(bq)).astype(np.float32) * scale).astype(
        np.float32
    )
    bk_p = np.asarray(bk).astype(np.float32)
    bk_p = _headmajor_b(bk_p)
    # fold bv through attention (softmax weights sum to 1) into bo:
    # bo' = bo + Wo @ bv_headmajor
    bv_hm = _headmajor_b(np.asarray(bv).astype(np.float32))
    bo_p = np.asarray(bo).astype(np.float32) + np.asarray(Wo).astype(
        np.float32
    ) @ bv_hm
    bcols = np.zeros((128, 19), dtype=np.float32)
    for j in range(NC_CHUNKS):
        bcols[:, j] = bq_s[j * 128 : (j + 1) * 128]
        bcols[:, 6 + j] = bk_p[j * 128 : (j + 1) * 128]
        bcols[:, 12 + j] = bo_p[j * 128 : (j + 1) * 128]
    bcols[:, 18] = EM_BIAS

    shared = {
        "wqt": wqt,
        "wkt": wkt,
        "wvt": wvt,
        "wot": wot,
        "bcols": np.ascontiguousarray(bcols),
    }
    in_maps = []
    for i in range(NCORES):
        sl = slice(i * NB, (i + 1) * NB)
        in_maps.append(
            dict(
                shared,
                x=np.ascontiguousarray(x[sl]),
                mask=np.ascontiguousarray(m[sl]),
            )
        )
    return in_maps


def kernel(**inputs):
    nc = _get_compiled()
    in_maps = prepare_in_maps(**inputs)
    res = run_bass_kernel_spmd(nc, in_maps, core_ids=list(range(NCORES)))
    y = np.concatenate([res.results[i]["y"] for i in range(NCORES)], axis=0)
    return y.reshape(B, C, 1, S)


# revision 35
# speedup vs baseline: 1.2623x; 1.0221x over previous
"""Trainium2 Bass kernel for nn_Attention_65455301591248.

Multi-head attention: B=32, C=768, H=12 heads, S=512, D=64.
  q/k/v = W{q,k,v} @ x + b   (1x1 conv == channel GEMM), head-minor channel
  scores[k,h,q] = (q.k)/sqrt(D) + mask[k,q];  softmax over k
  attn = w @ v; concat head-major; out = Wo @ attn + bo

Sharding: pure data parallel over batch - 4 batches per core x 8 cores,
no collectives.

Per-core kernel strategy (v2):
  - Host pre-transposes weights (lhsT layout) and permutes Q/K/V output
    channels head-major (c' = h*64+d).  Wq/bq pre-scaled by 1/8.
    bv folded into bo on host (softmax weights sum to 1, so +bv passes
    through attention unchanged): bo' = bo + Wo @ bv_hm.  Exact.
  - Scores for a head PAIR go into one [128,1024] PSUM tile (2 banks):
    even head -> cols 0:512 / rows 0:63 of the PE array, odd head ->
    cols 512:1024 / rows 64:127, explicit tile_position so the two
    K=64 matmuls run CONCURRENTLY (row tiling).
  - One exp per (pair,kc) over [128,1024] on ACT; one DVE multiply by
    em2 (mask exp, column-duplicated so the operand is contiguous).
    em is scaled by 2^-6 so fp16 reciprocals / weights stay in normal
    range; the scale cancels between numerator and denominator.
  - attn = V^T-slab @ w with a ones column per head: PSUM row 64
    accumulates the softmax denominator for free.  Eviction writes all
    heads of a half-batch into one [65,3072] tile, so ONE gather DMA
    collects the 6 denominator rows; reciprocal via
    reciprocal_approx_fast (fp32); one scatter DMA -> partition-0
    staging; gpsimd partition_broadcast + fp16 2x-mode DVE multiplies
    normalize into the head-major concat buffer.
  - Single large DMAs for weights / x / mask; prologue orders
    wq,wk,x,mask before wv,wo so score matmuls start early and the PE
    clock (HAM) stays warm.
"""

import numpy as np

try:
    import concourse.bass as bass  # noqa: F401
except ImportError:  # pragma: no cover
    import sys

    sys.path.insert(0, "/opt/trn_rl_repo")

import concourse.bass as bass
import concourse.tile as tile
from concourse import bacc, mybir
from concourse.bass_utils import run_bass_kernel_spmd

B, C, H, S, D = 32, 768, 12, 512, 64
NCORES = 8
NB = B // NCORES  # batches per core
F16 = mybir.dt.float16
F32 = mybir.dt.float32
NC_CHUNKS = C // 128  # 6
NK_CHUNKS = S // 128  # 4
VROW = H * (D + 1)  # 780: per-head 64 v columns + 1 ones column
EM_BIAS = -6.0 * float(np.log(2.0))  # exp(mask)*2^-6

_COMPILED = None


def _build():
    """Build + compile the per-core Bass program (runs on each of 8 cores)."""
    nc = bacc.Bacc("TRN2", target_bir_lowering=False, debug=False)

    x_d = nc.dram_tensor("x", [NB, C, S], F16, kind="ExternalInput")
    m_d = nc.dram_tensor("mask", [NB, S, S], F16, kind="ExternalInput")
    wq_d = nc.dram_tensor("wqt", [C, C], F16, kind="ExternalInput")
    wk_d = nc.dram_tensor("wkt", [C, C], F16, kind="ExternalInput")
    wv_d = nc.dram_tensor("wvt", [C, C], F16, kind="ExternalInput")
    wo_d = nc.dram_tensor("wot", [C, C], F16, kind="ExternalInput")
    # packed per-partition biases: cols 0-5 bq/8, 6-11 bk, 12-17 bo',
    # col 18 = EM_BIAS constant
    bcol_d = nc.dram_tensor("bcols", [128, 19], F32, kind="ExternalInput")
    y_d = nc.dram_tensor("y", [NB, C, S], F16, kind="ExternalOutput")

    with tile.TileContext(nc) as tc:
        with (
            tc.tile_pool(name="wpool", bufs=1) as wpool,
            tc.tile_pool(name="const", bufs=1) as const,
            tc.tile_pool(name="xp", bufs=2) as xp,
            tc.tile_pool(name="qk", bufs=2) as qk,
            tc.tile_pool(name="vp", bufs=2) as vp,
            tc.tile_pool(name="mp", bufs=2) as mp,
            tc.tile_pool(name="wexp", bufs=2) as wexp,
            tc.tile_pool(name="stgp", bufs=2) as stgp,
            tc.tile_pool(name="cat", bufs=2) as cat,
            tc.tile_pool(name="op", bufs=2) as op,
            tc.tile_pool(name="rp", bufs=2) as rp,
            tc.tile_pool(name="ps_proj", bufs=2, space="PSUM") as ps_proj,
            tc.tile_pool(name="ps_s", bufs=2, space="PSUM") as ps_s,
            tc.tile_pool(name="ps_a", bufs=2, space="PSUM") as ps_a,
        ):
            # ---- persistent weights / constants -------------------------
            def load_w(w_d, name, chunked=False):
                t = wpool.tile([128, NC_CHUNKS * C], F16, tag=name, name=name)
                if chunked:
                    # per-chunk DMAs give fine-grained deps: the ki-th
                    # accumulation matmul only waits for chunk ki
                    for j in range(NC_CHUNKS):
                        nc.sync.dma_start(
                            out=t[:, j * C : (j + 1) * C],
                            in_=w_d.ap()[j * 128 : (j + 1) * 128, :],
                        )
                else:
                    nc.sync.dma_start(
                        out=t.rearrange("p (j c) -> p j c", c=C),
                        in_=w_d.ap().rearrange("(j p) c -> p j c", p=128),
                    )
                return t

            bcol = const.tile([128, 19], F32, tag="bcol")
            nc.sync.dma_start(out=bcol[:], in_=bcol_d.ap()[:, :])

            def wview(t, ki):
                return t[:, ki * C : (ki + 1) * C]

            # ---- per-batch input loads ---------------------------------
            def load_x(b, chunked=False):
                t = xp.tile([128, NC_CHUNKS * S], F16, tag="x", name=f"x{b}")
                if chunked:
                    for j in range(NC_CHUNKS):
                        nc.sync.dma_start(
                            out=t[:, j * S : (j + 1) * S],
                            in_=x_d.ap()[b, j * 128 : (j + 1) * 128, :],
                        )
                else:
                    nc.sync.dma_start(
                        out=t.rearrange("p (j s) -> p j s", s=S),
                        in_=x_d.ap()[b].rearrange("(j p) s -> p j s", p=128),
                    )
                return t

            def load_mask(b):
                t = mp.tile([128, NK_CHUNKS * S], F16, tag="mraw", name=f"m{b}")
                nc.sync.dma_start(
                    out=t.rearrange("p (kc q) -> p kc q", q=S),
                    in_=m_d.ap()[b].rearrange("(kc p) q -> p kc q", p=128),
                )
                return t

            def em_thunk(mt, em_out):
                # em2 = exp(mask)*2^-6, with each kc chunk duplicated so the
                # es multiply gets one contiguous [128,1024] operand per kc.
                def one():
                    e = mp.tile([128, 2 * NK_CHUNKS * S], F16, tag="em", name="em")
                    ev = e.rearrange("p (kc r q) -> p kc r q", r=2, q=S)
                    mv = mt.rearrange("p (kc q) -> p kc q", q=S)
                    for r in range(2):
                        nc.scalar.activation(
                            out=ev[:, :, r, :],
                            in_=mv[:],
                            func=mybir.ActivationFunctionType.Exp,
                            bias=bcol[:, 18:19],
                        )
                    em_out[0] = e

                return one

            # ---- projection groups -------------------------------------
            def qk_group(w_t, xt, bias_col, name, co, outs):
                ps = ps_proj.tile([128, S], F32, tag="proj", name="ps_p")
                for ki in range(NC_CHUNKS):
                    nc.tensor.matmul(
                        ps[:],
                        wview(w_t, ki)[:, co * 128 : (co + 1) * 128],
                        xt[:, ki * S : (ki + 1) * S],
                        start=(ki == 0),
                        stop=(ki == NC_CHUNKS - 1),
                    )
                dt = qk.tile([128, S], F16, tag=f"{name}{co}", name=f"{name}{co}")
                if co % 2 == 0:
                    nc.vector.tensor_scalar_add(
                        dt[:], ps[:], bcol[:, bias_col + co : bias_col + co + 1]
                    )
                else:
                    nc.scalar.activation(
                        out=dt[:],
                        in_=ps[:],
                        func=mybir.ActivationFunctionType.Identity,
                        bias=bcol[:, bias_col + co : bias_col + co + 1],
                    )
                outs[co] = dt

            def v_group(wv_t, xt, sc, half, v_out):
                # v^T projection chunk: out [s, c'] with per-head ones col
                if half == 0:
                    vt = vp.tile([128, VROW], F16, tag=f"v{sc}", name=f"v{sc}")
                    vv = vt.rearrange("p (h w) -> p h w", w=D + 1)
                    nc.vector.memset(vv[:, :, D : D + 1], 1.0)
                    v_out[sc] = vt
                else:
                    vt = v_out[sc]
                    vv = vt.rearrange("p (h w) -> p h w", w=D + 1)
                v_out[(sc, half)] = True
                hw = C // 2  # 384 = 6 heads
                ps = ps_proj.tile([128, hw], F32, tag="proj", name="ps_v")
                for ki in range(NC_CHUNKS):
                    nc.tensor.matmul(
                        ps[:],
                        xt[:, ki * S + sc * 128 : ki * S + (sc + 1) * 128],
                        wview(wv_t, ki)[:, half * hw : (half + 1) * hw],
                        start=(ki == 0),
                        stop=(ki == NC_CHUNKS - 1),
                    )
                if half == 0:
                    nc.scalar.activation(
                        out=vv[:, 0:6, 0:D],
                        in_=ps.rearrange("p (h w) -> p h w", w=D),
                        func=mybir.ActivationFunctionType.Copy,
                    )
                else:
                    nc.vector.tensor_copy(
                        vv[:, 6:12, 0:D],
                        ps.rearrange("p (h w) -> p h w", w=D),
                    )

            def qkv_thunks(wv_load, xt, q_out, k_out, v_out):
                th = []
                for co in range(NC_CHUNKS):
                    th.append(lambda co=co: qk_group(wq_t, xt, 0, "q", co, q_out))
                    th.append(lambda co=co: qk_group(wk_t, xt, 6, "k", co, k_out))
                for sc in range(NK_CHUNKS):
                    for half in range(2):
                        th.append(
                            lambda sc=sc, half=half: v_group(
                                wv_load[0], xt, sc, half, v_out
                            )
                        )
                return th

            def oproj_thunks(wo_load, b, cat_sb):
                def one(co):
                    ps = ps_proj.tile([128, S], F32, tag="proj", name="ps_o")
                    for ki in range(NC_CHUNKS):
                        nc.tensor.matmul(
                            ps[:],
                            wview(wo_load[0], ki)[:, co * 128 : (co + 1) * 128],
                            cat_sb[ki][:],
                            start=(ki == 0),
                            stop=(ki == NC_CHUNKS - 1),
                        )
                    ot = op.tile([128, S], F16, tag=f"o{co}", name="ot")
                    if co % 2 == 0:
                        nc.vector.tensor_scalar_add(
                            ot[:], ps[:], bcol[:, 12 + co : 13 + co]
                        )
                    else:
                        nc.scalar.activation(
                            out=ot[:],
                            in_=ps[:],
                            func=mybir.ActivationFunctionType.Identity,
                            bias=bcol[:, 12 + co : 13 + co],
                        )
                    nc.sync.dma_start(
                        out=y_d.ap()[b, co * 128 : (co + 1) * 128, :], in_=ot[:]
                    )

                return [lambda co=co: one(co) for co in range(NC_CHUNKS)]

            # ---- attention for one batch -------------------------------
            def attention(b, q_sb, k_sb, v_sb, em_t, work, last=False):
                # q_sb/k_sb/v_sb are dicts filled lazily by work thunks;
                # drain_until pulls work forward when an operand tile has
                # not been emitted yet (only matters for batch 0).
                cat_sb = []
                for j in range(NC_CHUNKS):
                    ct = cat.tile([128, S], F16, tag=f"c{j}", name=f"cat{j}")
                    cat_sb.append(ct)
                stg = [
                    stgp.tile([D + 1, 6 * S], F16, tag="stg0", name="stg0"),
                    stgp.tile([D + 1, 6 * S], F16, tag="stg1", name="stg1"),
                ]

                nwork = len(work)
                wi = 0

                def drain_until(cond):
                    nonlocal wi
                    while not cond():
                        assert wi < nwork, "work list exhausted before operand ready"
                        work[wi]()
                        wi += 1

                def emit_scores_pair(hp):
                    em = em_t[0]
                    es_tiles = []
                    for kc in range(NK_CHUNKS):
                        ps = ps_s.tile([128, 2 * S], F32, tag="spair", name="ps_sc")
                        for j in range(2):
                            po = j * D
                            nc.tensor.matmul(
                                ps[:, j * S : (j + 1) * S],
                                k_sb[hp][po : po + D, kc * 128 : (kc + 1) * 128],
                                q_sb[hp][po : po + D, :],
                                start=True,
                                stop=True,
                                tile_position=(po, 0),
                            )
                        es = wexp.tile([128, 2 * S], F16, tag=f"es{kc}", name="es")
                        nc.scalar.activation(
                            out=es[:],
                            in_=ps[:],
                            func=mybir.ActivationFunctionType.Exp,
                        )
                        nc.vector.tensor_mul(
                            es[:], es[:], em[:, kc * 2 * S : (kc + 1) * 2 * S]
                        )
                        es_tiles.append(es)
                    return es_tiles

                def emit_attn_pair(hp, es_tiles):
                    for j in range(2):
                        h = 2 * hp + j
                        psa = ps_a.tile([D + 1, S], F32, tag="attn", name="psa")
                        for kc in range(NK_CHUNKS):
                            nc.tensor.matmul(
                                psa[:],
                                v_sb[kc][:, h * (D + 1) : (h + 1) * (D + 1)],
                                es_tiles[kc][:, j * S : (j + 1) * S],
                                start=(kc == 0),
                                stop=(kc == NK_CHUNKS - 1),
                            )
                        dst = stg[h // 6][:, (h % 6) * S : (h % 6 + 1) * S]
                        if j == 0:
                            nc.vector.tensor_copy(dst, psa[:])
                        else:
                            nc.scalar.activation(
                                out=dst,
                                in_=psa[:],
                                func=mybir.ActivationFunctionType.Copy,
                            )

                def emit_norm_half(half):
                    sh = stg[half]
                    r12 = rp.tile([6, S], F16, tag="r12", name="r12")
                    nc.sync.dma_start(out=r12[:], in_=sh[D : D + 1, :])
                    r12f = rp.tile([6, S], F32, tag="r12f", name="r12f")
                    nc.vector.tensor_copy(r12f[:], r12[:])
                    rrf = rp.tile([6, S], F32, tag="rrf", name="rrf")
                    nc.vector.reciprocal_approx_fast(out=rrf[:], in_=r12f[:])
                    rr = rp.tile([6, S], F16, tag="rr", name="rr")
                    nc.scalar.activation(
                        out=rr[:],
                        in_=rrf[:],
                        func=mybir.ActivationFunctionType.Copy,
                    )
                    rbsrc = rp.tile([1, 6 * S], F16, tag="rbs", name="rbs")
                    nc.sync.dma_start(out=rbsrc[:], in_=rr[:])
                    for jj in range(6):
                        h = 6 * half + jj
                        hp, po = h // 2, (h % 2) * D
                        rb = rp.tile([D, S], F16, tag=f"rb{jj % 2}", name="rb")
                        nc.gpsimd.partition_broadcast(
                            rb[:], rbsrc[0:1, jj * S : (jj + 1) * S]
                        )
                        nc.vector.tensor_mul(
                            cat_sb[hp][po : po + D, :],
                            sh[0:D, jj * S : (jj + 1) * S],
                            rb[:],
                        )

                def emit_norm_pair(hp):
                    # DMA-free tail norm (last batch, pairs 3-5): PE
                    # broadcasts the denominator row (K=1 matmul from
                    # partition 64), reciprocal in place on PSUM, multiply.
                    sh = stg[1]
                    for j in range(2):
                        h = 2 * hp + j
                        col = (h - 6) * S
                        psrb = ps_a.tile([D, S], F32, tag="attn", name="psrb")
                        nc.tensor.matmul(
                            psrb[:],
                            ones65[D : D + 1, :],
                            sh[D : D + 1, col : col + S],
                            start=True,
                            stop=True,
                        )
                        nc.vector.reciprocal_approx_fast(
                            out=psrb[:], in_=psrb[:]
                        )
                        nc.vector.tensor_mul(
                            cat_sb[hp][j * D : (j + 1) * D, :],
                            sh[0:D, col : col + S],
                            psrb[:],
                        )

                def v_ready(hp):
                    need = [(kc, 0) for kc in range(NK_CHUNKS)]
                    if hp >= 3:
                        need += [(kc, 1) for kc in range(NK_CHUNKS)]
                    return all(k in v_sb for k in need)

                # spread the work thunks across the head pairs
                pend = None
                for hp in range(H // 2):
                    drain_until(
                        lambda: hp in q_sb and hp in k_sb and em_t[0] is not None
                    )
                    es_tiles = emit_scores_pair(hp)
                    if pend is not None:
                        drain_until(lambda: v_ready(pend[0]))
                        emit_attn_pair(pend[0], pend[1])
                        if pend[0] == 2:
                            emit_norm_half(0)
                        elif last and pend[0] >= 3:
                            emit_norm_pair(pend[0])
                    pend = (hp, es_tiles)
                    target = (hp + 1) * nwork // (H // 2)
                    while wi < target:
                        work[wi]()
                        wi += 1
                emit_attn_pair(pend[0], pend[1])
                if last:
                    emit_norm_pair(5)
                else:
                    emit_norm_half(1)
                while wi < nwork:
                    work[wi]()
                    wi += 1
                return cat_sb

            # ---- prologue: batch 0 -------------------------------------
            # Dummy matmuls on a zeroed tile bridge the DMA-startup window
            # so the PE clock (HAM) is already warm when real work lands.
            dummy = const.tile([128, S], F16, tag="dummy")
            nc.vector.memset(dummy[:], 0.0)
            # ones column at partition 64 for the PE-broadcast tail norm
            ones65 = const.tile([D + 1, D], F16, tag="ones65")
            nc.vector.memset(ones65[:], 1.0)
            for i in range(36):
                psd = ps_proj.tile([128, S], F32, tag="proj", name="ps_d")
                nc.tensor.matmul(
                    psd[:], dummy[:, 0:128], dummy[:], start=True, stop=True
                )
                if i == 35:
                    nc.vector.tensor_copy(dummy[:, 0:1], psd[:, 0:1])

            # DMA order: wq/x interleaved per chunk (matmul ki only waits
            # for chunk ki), then wk, mask0, wv, wo.
            wq_t = wpool.tile([128, NC_CHUNKS * C], F16, tag="wq", name="wq")
            xt0 = xp.tile([128, NC_CHUNKS * S], F16, tag="x", name="x0")
            for j in range(NC_CHUNKS):
                nc.sync.dma_start(
                    out=wq_t[:, j * C : (j + 1) * C],
                    in_=wq_d.ap()[j * 128 : (j + 1) * 128, :],
                )
                nc.sync.dma_start(
                    out=xt0[:, j * S : (j + 1) * S],
                    in_=x_d.ap()[0, j * 128 : (j + 1) * 128, :],
                )
            wk_t = load_w(wk_d, "wk", chunked=True)
            mt0 = load_mask(0)
            wv_load, wo_load = [None], [None]

            def loadwv():
                wv_load[0] = load_w(wv_d, "wv")

            def loadwo():
                wo_load[0] = load_w(wo_d, "wo")

            em_cur = [None]
            q_cur, k_cur, v_cur = {}, {}, {}
            # emit first q/k groups so scores pair 0 can start early
            qk_group(wq_t, xt0, 0, "q", 0, q_cur)
            qk_group(wk_t, xt0, 6, "k", 0, k_cur)

            def qg(co):
                return lambda: qk_group(wq_t, xt0, 0, "q", co, q_cur)

            def kg(co):
                return lambda: qk_group(wk_t, xt0, 6, "k", co, k_cur)

            def vg(sc, half):
                return lambda: v_group(wv_load[0], xt0, sc, half, v_cur)

            work0 = [
                em_thunk(mt0, em_cur),
                qg(1), kg(1), qg(2), kg(2),
                loadwv,
                vg(0, 0), vg(1, 0), vg(2, 0), vg(3, 0),
                qg(3), kg(3), qg(4), kg(4),
                vg(0, 1), vg(1, 1), vg(2, 1), vg(3, 1),
                qg(5), kg(5),
                loadwo,
            ]

            prev_cat = None
            work = work0
            for b in range(NB):
                em_next = [None]
                q_next, k_next, v_next = {}, {}, {}
                if b + 1 < NB:
                    xt_next = load_x(b + 1)
                    mt_next = load_mask(b + 1)
                    work.append(em_thunk(mt_next, em_next))
                if prev_cat is not None:
                    work += oproj_thunks(wo_load, b - 1, prev_cat)
                if b + 1 < NB:
                    work += qkv_thunks(wv_load, xt_next, q_next, k_next, v_next)
                prev_cat = attention(
                    b, q_cur, k_cur, v_cur, em_cur, work, last=(b == NB - 1)
                )
                em_cur, q_cur, k_cur, v_cur = em_next, q_next, k_next, v_next
                work = []

            # final oproj, ki-split: chunks 0-4 are normed (half 0 + pair
            # norms 3,4) well before the pair-5 chain lands, so the first
            # 30 matmuls execute during that chain; ki=5 closes each group.
            pso = []
            for _ in range(2):
                pt = ps_s.tile([128, 2 * S], F32, tag="spair", name="ps_fo")
                pso.append(pt[:, 0:S])
                pso.append(pt[:, S : 2 * S])
            for _ in range(2):
                pt = ps_proj.tile([128, S], F32, tag="proj", name="ps_fo2")
                pso.append(pt[:])
            wo_t = wo_load[0]
            for co in range(NC_CHUNKS):
                for ki in range(5):
                    nc.tensor.matmul(
                        pso[co],
                        wview(wo_t, ki)[:, co * 128 : (co + 1) * 128],
                        prev_cat[ki][:],
                        start=(ki == 0),
                        stop=False,
                    )
            for co in range(NC_CHUNKS):
                for ki in range(5, NC_CHUNKS):
                    nc.tensor.matmul(
                        pso[co],
                        wview(wo_t, ki)[:, co * 128 : (co + 1) * 128],
                        prev_cat[ki][:],
                        start=False,
                        stop=(ki == NC_CHUNKS - 1),
                    )
                ot = op.tile([128, S], F16, tag=f"o{co}", name="ot")
                if co % 2 == 0:
                    nc.vector.tensor_scalar_add(
                        ot[:], pso[co], bcol[:, 12 + co : 13 + co]
                    )
                    nc.sync.dma_start(
                        out=y_d.ap()[NB - 1, co * 128 : (co + 1) * 128, :],
                        in_=ot[:],
                    )
                else:
                    nc.scalar.activation(
                        out=ot[:],
                        in_=pso[co],
                        func=mybir.ActivationFunctionType.Identity,
                        bias=bcol[:, 12 + co : 13 + co],
                    )
                    # issue from the ACT queue: SyncE is draining its own
                    # semaphores at the tail
                    nc.scalar.dma_start(
                        out=y_d.ap()[NB - 1, co * 128 : (co + 1) * 128, :],
                        in_=ot[:],
                    )

    nc.compile()
    return nc


def _get_compiled():
    global _COMPILED
    if _COMPILED is None:
        _COMPILED = _build()
    return _COMPILED


def _headmajor(wT):
    """Permute the output-channel axis of a transposed weight from the
    reference's head-minor order (c = d*H + h) to head-major (c' = h*D + d)."""
    return np.ascontiguousarray(
        wT.reshape(C, D, H).transpose(0, 2, 1).reshape(C, C)
    )


def _headmajor_b(bv):
    return np.ascontiguousarray(bv.reshape(D, H).T.reshape(C))


def prepare_in_maps(hidden_state, mask, Wq, bq, Wk, bk, Wv, bv, Wo, bo):
    x = np.asarray(hidden_state).reshape(B, C, S)
    m = np.asarray(mask).reshape(B, S, S)
    scale = np.float32(D**-0.5)

    wqt = np.ascontiguousarray(
        (_headmajor(np.asarray(Wq).T).astype(np.float32) * scale).astype(np.float16)
    )
    wkt = _headmajor(np.asarray(Wk).T)
    wvt = _headmajor(np.asarray(Wv).T)
    wot = np.ascontiguousarray(np.asarray(Wo).T)

    bq_s = (_headmajor_b(np.asarray(bq)).astype(np.float32) * scale).astype(
        np.float32
    )
    bk_p = np.asarray(bk).astype(np.float32)
    bk_p = _headmajor_b(bk_p)
    # fold bv through attention (softmax weights sum to 1) into bo:
    # bo' = bo + Wo @ bv_headmajor
    bv_hm = _headmajor_b(np.asarray(bv).astype(np.float32))
    bo_p = np.asarray(bo).astype(np.float32) + np.asarray(Wo).astype(
        np.float32
    ) @ bv_hm
    bcols = np.zeros((128, 19), dtype=np.float32)
    for j in range(NC_CHUNKS):
        bcols[:, j] = bq_s[j * 128 : (j + 1) * 128]
        bcols[:, 6 + j] = bk_p[j * 128 : (j + 1) * 128]
        bcols[:, 12 + j] = bo_p[j * 128 : (j + 1) * 128]
    bcols[:, 18] = EM_BIAS

    shared = {
        "wqt": wqt,
        "wkt": wkt,
        "wvt": wvt,
        "wot": wot,
        "bcols": np.ascontiguousarray(bcols),
    }
    in_maps = []
    for i in range(NCORES):
        sl = slice(i * NB, (i + 1) * NB)
        in_maps.append(
            dict(
                shared,
                x=np.ascontiguousarray(x[sl]),
                mask=np.ascontiguousarray(m[sl]),
            )
        )
    return in_maps


def kernel(**inputs):
    nc = _get_compiled()
    in_maps = prepare_in_maps(**inputs)
    res = run_bass_kernel_spmd(nc, in_maps, core_ids=list(range(NCORES)))
    y = np.concatenate([res.results[i]["y"] for i in range(NCORES)], axis=0)
    return y.reshape(B, C, 1, S)
